# revision 4
# baseline (speedup 1.0000x reference)
"""Trainium2 Bass kernel for nn_Decoder (sparse top-8 attention decoder layer).

Contract: kernel(**inputs) takes the FULL unsharded inputs (B=2, S=2048,
D=1024, H=16 heads, top-8 sparse causal attention + ReZero FFN) and returns
the full [2, 2048, 1024] fp32 output.

Sharding: sequence-parallel over 8 cores, no collectives. Each core owns 4
query tiles of 128 rows from one batch, one tile from each causal-length
quartile so the SPMD program is uniform and balanced: core i (b = i//4,
m = i%4) owns absolute q-tiles {m, m+4, m+8, m+12}; q-tile m+4s runs in
"slot" s scanning a key window of 512*(s+1) keys (true causal window is
smaller; the remainder is masked additively). Every core redundantly
computes K/V projections for all 2048 keys of its batch.

Numerics: Q/K projections and Q.K^T use a 3-term bf16 hi/lo split
(error ~2^-17) because top-8 selection is sensitive to score noise near the
8th/9th-largest boundary (plain bf16 flips ~5% of rows and fails).
V/Wo/FFN run in bf16 with fp32 accumulation. Softmax runs on the 8
surviving scores only (exp of masked scores underflows to exactly 0,
matching the reference's -10000 masking); attn@V is a dma_gather of the 8
selected V rows per query plus a small weighted reduction instead of a
dense [S,S] @ [S,DK] matmul.
"""

import math
from contextlib import ExitStack

import ml_dtypes
import numpy as np

B, S, D, H, TOPK, DK = 2, 2048, 1024, 16, 8, 64
NC = 8          # cores
SQ = 512        # query rows per core (4 tiles of 128)
KE = 2048       # key window per core (uniform)
NT = 4          # q-tile slots per core; slot s scans 512*(s+1) keys
NEGBIG = -100000.0

_compiled = None


def _bf16_split(a):
    hi = a.astype(ml_dtypes.bfloat16)
    lo = (a - hi.astype(np.float32)).astype(ml_dtypes.bfloat16)
    return hi, lo


def _build_program():
    import concourse.tile as tile
    from concourse import bacc, mybir

    f32 = mybir.dt.float32
    bf16 = mybir.dt.bfloat16
    u16 = mybir.dt.uint16

    nc = bacc.Bacc("TRN2", target_bir_lowering=False, debug=False, num_devices=NC)

    def din(name, shape, dt):
        return nc.dram_tensor(name, shape, dt, kind="ExternalInput").ap()

    aps = {
        "xk_hi": din("xk_hi", [D, KE], bf16),
        "xk_lo": din("xk_lo", [D, KE], bf16),
        "xq_hi": din("xq_hi", [D, SQ], bf16),
        "xq_lo": din("xq_lo", [D, SQ], bf16),
        "wq_hi": din("wq_hi", [D, D], bf16),
        "wq_lo": din("wq_lo", [D, D], bf16),
        "wk_hi": din("wk_hi", [D, D], bf16),
        "wk_lo": din("wk_lo", [D, D], bf16),
        "bq_row": din("bq_row", [1, D], bf16),
        "bk_row": din("bk_row", [1, D], bf16),
        "wv": din("wv", [D, D], bf16),
        "bv_row": din("bv_row", [1, D], bf16),
        "wo": din("wo", [D, D], bf16),
        "wo_b": din("wo_b", [1, D], bf16),
        "w1": din("w1", [D, 4 * D], bf16),
        "b1t": din("b1t", [128, 32], f32),
        "w2": din("w2", [4 * D, D], bf16),
        "b2_row": din("b2_row", [1, D], bf16),
        "maskd": din("maskd", [NT, 128, 512], f32),
        "ident": din("ident", [128, 128], bf16),
        "y": nc.dram_tensor("y", [SQ, D], f32, kind="ExternalOutput").ap(),
        "v_dram": nc.dram_tensor("v_dram", [KE, D], f32).ap(),
        "idx_dram": nc.dram_tensor("idx_dram", [H, NT, 128, 8], u16).ap(),
    }

    with tile.TileContext(nc) as tc:
        _emit(nc, tc, mybir, aps)
    nc.compile()
    return nc


def _emit(nc, tc, mybir, t):
    f32 = mybir.dt.float32
    bf16 = mybir.dt.bfloat16
    u16 = mybir.dt.uint16
    i16 = mybir.dt.int16
    AF = mybir.ActivationFunctionType
    OP = mybir.AluOpType

    y_d, v_dram, idx_dram = t["y"], t["v_dram"], t["idx_dram"]

    with ExitStack() as ctx:
        # ---------------- constants (live whole kernel) -----------------
        const = ctx.enter_context(tc.tile_pool(name="const", bufs=1))
        ones = const.tile([1, 512], bf16)
        nc.vector.memset(ones[:], 1.0)
        identt = const.tile([128, 128], bf16)
        nc.sync.dma_start(identt[:], t["ident"][:, :])
        brow = {}
        for nm, key in [("bq", "bq_row"), ("bk", "bk_row"), ("bv", "bv_row"),
                        ("wo_b", "wo_b"), ("b2", "b2_row")]:
            r = const.tile([1, D], bf16, tag=f"bias_{nm}")
            nc.sync.dma_start(r[:], t[key][:, :])
            brow[nm] = r
        b1t = const.tile([128, 32], f32)
        nc.sync.dma_start(b1t[:], t["b1t"][:, :])
        maskd = const.tile([128, NT * 512], f32)
        nc.sync.dma_start(maskd[:].rearrange("p (s c) -> p s c", s=NT),
                          t["maskd"].rearrange("s p c -> p s c"))

        # ctx outputs of attention, consumed by Wo phase
        ctxp = ctx.enter_context(tc.tile_pool(name="ctxT", bufs=1))
        ctxT = [ctxp.tile([128, SQ], bf16, tag=f"ctxT{hp}", name=f"ctxT{hp}")
                for hp in range(8)]

        with tc.tile_pool(name="x", bufs=1) as xpool:
            xkh, xkl, xqh, xql = [], [], [], []
            for c in range(8):
                sl = slice(c * 128, (c + 1) * 128)
                th = xpool.tile([128, KE], bf16, tag=f"xkh{c}")
                nc.sync.dma_start(th[:], t["xk_hi"][sl, :])
                xkh.append(th)
                tl = xpool.tile([128, KE], bf16, tag=f"xkl{c}")
                nc.sync.dma_start(tl[:], t["xk_lo"][sl, :])
                xkl.append(tl)
                qh = xpool.tile([128, SQ], bf16, tag=f"xqh{c}")
                nc.sync.dma_start(qh[:], t["xq_hi"][sl, :])
                xqh.append(qh)
                ql = xpool.tile([128, SQ], bf16, tag=f"xql{c}")
                nc.sync.dma_start(ql[:], t["xq_lo"][sl, :])
                xql.append(ql)

            # ------------ phase V: v = x @ Wv.T + bv -> v_dram ----------
            with tc.tile_pool(name="wvp", bufs=1) as wvp, \
                 tc.tile_pool(name="vps", bufs=4, space="PSUM") as vps, \
                 tc.tile_pool(name="vsb", bufs=4) as vsbp:
                wvt = []
                for c in range(8):
                    w = wvp.tile([128, D], bf16, tag=f"wv{c}")
                    nc.sync.dma_start(w[:], t["wv"][c * 128:(c + 1) * 128, :])
                    wvt.append(w)
                for rt in range(16):
                    for oc in range(2):
                        ps = vps.tile([128, 512], f32)
                        for c in range(8):
                            nc.tensor.matmul(
                                ps[:], xkh[c][:, rt * 128:(rt + 1) * 128],
                                wvt[c][:, oc * 512:(oc + 1) * 512],
                                start=(c == 0), stop=False)
                        nc.tensor.matmul(
                            ps[:], ones[0:1, 0:128],
                            brow["bv"][0:1, oc * 512:(oc + 1) * 512],
                            start=False, stop=True)
                        vs = vsbp.tile([128, 512], f32)
                        nc.scalar.copy(vs[:], ps[:])
                        nc.sync.dma_start(
                            v_dram[rt * 128:(rt + 1) * 128,
                                   oc * 512:(oc + 1) * 512], vs[:])

            # ------------ attention loop over head-pairs ----------------
            with tc.tile_pool(name="att", bufs=2) as att, \
                 tc.tile_pool(name="attps", bufs=2, space="PSUM") as attps, \
                 tc.tile_pool(name="scps", bufs=3, space="PSUM") as scps, \
                 tc.tile_pool(name="ctps", bufs=2, space="PSUM") as ctps, \
                 tc.tile_pool(name="scores", bufs=2) as scores_p, \
                 tc.tile_pool(name="small", bufs=4) as small, \
                 tc.tile_pool(name="gather", bufs=3) as gat:
                for hp in range(8):
                    cs, ce = hp * 128, (hp + 1) * 128
                    wkh = att.tile([128, 1024], bf16, tag="wkh")
                    wkl = att.tile([128, 1024], bf16, tag="wkl")
                    wqh = att.tile([128, 1024], bf16, tag="wqh")
                    wql = att.tile([128, 1024], bf16, tag="wql")
                    for c in range(8):
                        sl = slice(c * 128, (c + 1) * 128)
                        dsl = slice(c * 128, (c + 1) * 128)
                        nc.sync.dma_start(wkh[:, dsl], t["wk_hi"][sl, cs:ce])
                        nc.sync.dma_start(wkl[:, dsl], t["wk_lo"][sl, cs:ce])
                        nc.sync.dma_start(wqh[:, dsl], t["wq_hi"][sl, cs:ce])
                        nc.sync.dma_start(wql[:, dsl], t["wq_lo"][sl, cs:ce])

                    # K projection (3-pass hi/lo split + bias)
                    khi = att.tile([128, KE], bf16, tag="khi")
                    klo = att.tile([128, KE], bf16, tag="klo")
                    for kc in range(4):
                        ksl = slice(kc * 512, (kc + 1) * 512)
                        ps = attps.tile([128, 512], f32, tag="projps")
                        first = True
                        for wt, xt in ((wkh, xkh), (wkl, xkh), (wkh, xkl)):
                            for c in range(8):
                                nc.tensor.matmul(
                                    ps[:], wt[:, c * 128:(c + 1) * 128],
                                    xt[c][:, ksl], start=first, stop=False)
                                first = False
                        nc.tensor.matmul(ps[:], brow["bk"][0:1, cs:ce],
                                         ones[0:1, 0:512], start=False, stop=True)
                        nc.scalar.copy(khi[:, ksl], ps[:])
                        nc.vector.tensor_sub(klo[:, ksl], ps[:], khi[:, ksl])

                    # Q projection
                    qhi = att.tile([128, SQ], bf16, tag="qhi")
                    qlo = att.tile([128, SQ], bf16, tag="qlo")
                    ps = attps.tile([128, 512], f32, tag="projps")
                    first = True
                    for wt, xt in ((wqh, xqh), (wql, xqh), (wqh, xql)):
                        for c in range(8):
                            nc.tensor.matmul(
                                ps[:], wt[:, c * 128:(c + 1) * 128], xt[c][:, :],
                                start=first, stop=False)
                            first = False
                    nc.tensor.matmul(ps[:], brow["bq"][0:1, cs:ce],
                                     ones[0:1, 0:512], start=False, stop=True)
                    nc.scalar.copy(qhi[:, :], ps[:])
                    nc.vector.tensor_sub(qlo[:, :], ps[:], qhi[:, :])

                    for h2 in range(2):
                        h = 2 * hp + h2
                        hsl = slice(64 * h2, 64 * h2 + 64)
                        for s in range(NT):
                            L = 512 * (s + 1)
                            qsl = slice(128 * s, 128 * (s + 1))
                            sc = scores_p.tile([128, KE], f32, tag="scores")
                            for kc in range(s + 1):
                                ksl = slice(kc * 512, (kc + 1) * 512)
                                sp = scps.tile([128, 512], f32, tag="scoreps")
                                nc.tensor.matmul(sp[:], qhi[hsl, qsl],
                                                 khi[hsl, ksl],
                                                 start=True, stop=False)
                                nc.tensor.matmul(sp[:], qhi[hsl, qsl],
                                                 klo[hsl, ksl],
                                                 start=False, stop=False)
                                nc.tensor.matmul(sp[:], qlo[hsl, qsl],
                                                 khi[hsl, ksl],
                                                 start=False, stop=True)
                                if kc == s:
                                    nc.vector.tensor_add(
                                        sc[:, ksl], sp[:],
                                        maskd[:, s * 512:(s + 1) * 512])
                                else:
                                    nc.scalar.copy(sc[:, ksl], sp[:])
                            m8 = small.tile([128, 8], f32, tag="m8")
                            nc.vector.max(m8[:], sc[:, 0:L])
                            idx = small.tile([128, 8], u16, tag="idx")
                            nc.vector.max_index(idx[:], m8[:], sc[:, 0:L])
                            w8 = small.tile([128, 8], f32, tag="w8")
                            z = small.tile([128, 1], f32, tag="z")
                            nc.scalar.activation(w8[:], m8[:], AF.Exp,
                                                 scale=1.0 / math.sqrt(DK),
                                                 accum_out=z[:])
                            rz = small.tile([128, 1], f32, tag="rz")
                            nc.vector.reciprocal(rz[:], z[:])
                            wn = small.tile([128, 8], f32, tag="wn")
                            nc.vector.tensor_scalar_mul(wn[:], w8[:], rz[:])
                            # index relayout: SBUF -> DRAM -> wrapped SBUF x8
                            idd = idx_dram[h, s, :, :]
                            nc.sync.dma_start(idd, idx[:])
                            wrap = small.tile([128, 64], i16, tag="wrap")
                            src = idx_dram[h, s].rearrange(
                                "(g r) j -> r j g", r=16).bitcast(i16)
                            for grp in range(8):
                                dst = wrap[16 * grp:16 * (grp + 1), :].rearrange(
                                    "r (j g) -> r j g", j=8)
                                nc.sync.dma_start(dst, src)
                            gth = gat.tile([128, 8, 64], f32, tag="gth")
                            nc.gpsimd.dma_gather(
                                out_ap=gth[:],
                                in_ap=v_dram[:, 64 * h:64 * (h + 1)],
                                idxs_ap=wrap[:],
                                num_idxs=1024, num_idxs_reg=1024,
                                elem_size=64, elem_step=D,
                                queue_num=0,
                            )
                            prod = gat.tile([128, 8, 64], f32, tag="prod")
                            nc.vector.tensor_mul(
                                prod[:], gth[:],
                                wn[:].unsqueeze(2).broadcast_to([128, 8, 64]))
                            cx = small.tile([128, 64], f32, tag="cx")
                            nc.vector.tensor_reduce(
                                cx[:], prod[:].rearrange("p j d -> p d j"),
                                axis=mybir.AxisListType.X, op=OP.add)
                            cxb = small.tile([128, 64], bf16, tag="cxb")
                            nc.scalar.copy(cxb[:], cx[:])
                            ct = ctps.tile([64, 128], bf16, tag="ctp")
                            nc.tensor.transpose(ct[:], cxb[:], identt[:])
                            nc.scalar.copy(ctxT[hp][hsl, qsl], ct[:])

        # ---------------- Wo: hT = (2 g1)(ctx Wo.T + bo), transposed ----
        hTp = ctx.enter_context(tc.tile_pool(name="hT", bufs=1))
        hT = []
        with tc.tile_pool(name="wop", bufs=1) as wop, \
             tc.tile_pool(name="wops", bufs=4, space="PSUM") as wops:
            wot = []
            for c in range(8):
                w = wop.tile([128, D], bf16, tag=f"wo{c}")
                nc.sync.dma_start(w[:], t["wo"][c * 128:(c + 1) * 128, :])
                wot.append(w)
            for ot in range(8):
                ps = wops.tile([128, 512], f32)
                for c in range(8):
                    nc.tensor.matmul(ps[:], wot[c][:, ot * 128:(ot + 1) * 128],
                                     ctxT[c][:, :], start=(c == 0), stop=False)
                nc.tensor.matmul(
                    ps[:], brow["wo_b"][0:1, ot * 128:(ot + 1) * 128],
                    ones[0:1, 0:512], start=False, stop=True)
                ht = hTp.tile([128, SQ], bf16, tag=f"hT{ot}")
                nc.scalar.copy(ht[:], ps[:])
                hT.append(ht)

        # ---------------- FFN mm1 + gelu --------------------------------
        gTp = ctx.enter_context(tc.tile_pool(name="gT", bufs=1))
        gT = []
        with tc.tile_pool(name="w1p", bufs=1) as w1p, \
             tc.tile_pool(name="f1ps", bufs=4, space="PSUM") as f1ps:
            w1t = []
            for c in range(8):
                w = w1p.tile([128, 4 * D], bf16, tag=f"w1{c}")
                nc.sync.dma_start(w[:], t["w1"][c * 128:(c + 1) * 128, :])
                w1t.append(w)
            for ft in range(32):
                ps = f1ps.tile([128, 512], f32)
                for c in range(8):
                    nc.tensor.matmul(ps[:], w1t[c][:, ft * 128:(ft + 1) * 128],
                                     hT[c][:, :], start=(c == 0), stop=(c == 7))
                g = gTp.tile([128, SQ], bf16, tag=f"gT{ft}")
                nc.scalar.activation(g[:], ps[:], AF.Gelu,
                                     bias=b1t[:, ft:ft + 1], scale=1.0)
                gT.append(g)

        # ---------------- FFN mm2 + bias + out --------------------------
        with tc.tile_pool(name="w2p", bufs=1) as w2p, \
             tc.tile_pool(name="yps", bufs=4, space="PSUM") as yps, \
             tc.tile_pool(name="ysb", bufs=4) as ysbp:
            w2t = []
            for fc in range(32):
                w = w2p.tile([128, D], bf16, tag=f"w2{fc}")
                nc.sync.dma_start(w[:], t["w2"][fc * 128:(fc + 1) * 128, :])
                w2t.append(w)
            for qt in range(4):
                for oc in range(2):
                    ps = yps.tile([128, 512], f32)
                    for fc in range(32):
                        nc.tensor.matmul(
                            ps[:], gT[fc][:, qt * 128:(qt + 1) * 128],
                            w2t[fc][:, oc * 512:(oc + 1) * 512],
                            start=(fc == 0), stop=False)
                    nc.tensor.matmul(
                        ps[:], ones[0:1, 0:128],
                        brow["b2"][0:1, oc * 512:(oc + 1) * 512],
                        start=False, stop=True)
                    ys = ysbp.tile([128, 512], f32)
                    nc.scalar.copy(ys[:], ps[:])
                    nc.sync.dma_start(
                        y_d[qt * 128:(qt + 1) * 128, oc * 512:(oc + 1) * 512],
                        ys[:])


def _prep_inputs(x, Wq, bq, Wk, bk, Wv, bv, Wo, bo, g1, W1, b1, W2, b2, g2):
    f32 = np.float32
    bf = ml_dtypes.bfloat16
    x = np.asarray(x, f32)
    g1 = float(np.asarray(g1))
    g2 = float(np.asarray(g2))

    wq_hi, wq_lo = _bf16_split(np.ascontiguousarray(np.asarray(Wq, f32).T))
    wk_hi, wk_lo = _bf16_split(np.ascontiguousarray(np.asarray(Wk, f32).T))
    shared = {
        "wq_hi": wq_hi, "wq_lo": wq_lo, "wk_hi": wk_hi, "wk_lo": wk_lo,
        "bq_row": np.asarray(bq, f32).reshape(1, D).astype(bf),
        "bk_row": np.asarray(bk, f32).reshape(1, D).astype(bf),
        "wv": np.ascontiguousarray(np.asarray(Wv, f32).T).astype(bf),
        "bv_row": np.asarray(bv, f32).reshape(1, D).astype(bf),
        "wo": np.ascontiguousarray(2 * g1 * np.asarray(Wo, f32).T).astype(bf),
        "wo_b": (2 * g1 * np.asarray(bo, f32)).reshape(1, D).astype(bf),
        "w1": np.ascontiguousarray(np.asarray(W1, f32).T).astype(bf),
        "b1t": np.asarray(b1, f32).reshape(32, 128).T.copy(),
        "w2": np.ascontiguousarray(2 * g2 * np.asarray(W2, f32).T).astype(bf),
        "b2_row": (2 * g2 * np.asarray(b2, f32)).reshape(1, D).astype(bf),
        "ident": np.eye(128, dtype=bf),
    }
    xsplit = {b: _bf16_split(np.ascontiguousarray(x[b].T)) for b in range(B)}

    in_maps = []
    for i in range(NC):
        b, m = i // 4, i % 4
        xh, xl = xsplit[b]
        tiles = [m + 4 * s for s in range(NT)]
        qcols = np.concatenate(
            [np.arange(128 * tt, 128 * (tt + 1)) for tt in tiles])
        mask = np.zeros((NT, 128, 512), f32)
        for s, tt in enumerate(tiles):
            key = 512 * s + np.arange(512)[None, :]
            qabs = 128 * tt + np.arange(128)[:, None]
            mask[s][key > qabs] = NEGBIG
        im = dict(shared)
        im.update({
            "xk_hi": xh, "xk_lo": xl,
            "xq_hi": np.ascontiguousarray(xh[:, qcols]),
            "xq_lo": np.ascontiguousarray(xl[:, qcols]),
            "maskd": mask,
        })
        in_maps.append(im)
    return in_maps


def kernel(**inputs):
    global _compiled
    from concourse.bass_utils import run_bass_kernel_spmd

    if _compiled is None:
        _compiled = _build_program()

    in_maps = _prep_inputs(**inputs)
    res = run_bass_kernel_spmd(_compiled, in_maps, core_ids=list(range(NC)))

    out = np.empty((B, S, D), np.float32)
    for i in range(NC):
        b, m = i // 4, i % 4
        yc = res.results[i]["y"]
        for s in range(NT):
            tt = m + 4 * s
            out[b, 128 * tt:128 * (tt + 1), :] = yc[128 * s:128 * (s + 1), :]
    return out
